# revision 5
# baseline (speedup 1.0000x reference)
"""Distributed Trainium2 kernel for nn_Attention_2654289789382 (sparse_attention).

Math (reference):
    sigma = sigmoid(x @ W_sigma + b_sigma)           (b, h, n)
    den_i = exp(sigma)+1 ;  r_i = 1/den_i = sigmoid(-sigma)   in (0.2689, 0.5)
    prior[i,j] = softmax_j(-|i-j| * r_i)
    out = (prior @ v) reshaped @ W_out + b_out,  v = x @ W_v

Structure exploited:
  * r_i >= 0.2689  =>  banded attention, half-width 64: per 128-row i-block
    only 2 j-tiles of 128 (at +-64 offsets) contribute.
  * softmax denominator in closed form (two-sided geometric series):
        den_i = 1 + (2z - z^(i+1) - z^(n-i)) / (1-z),  z = exp(-r_i)
  * No Sigmoid activation table: sigma and r = 1/den via Exp + DVE
    reciprocal only; Exp table preloaded at t=0 by a dummy activation.
  * HAM warmup: dummy matmuls at t=0 so real GEMMs run at 2.4 GHz.
  * Inputs host-packed into few fat DRAM tensors, loaded with one DMA
    each across 3 issuing engines (sync / scalar HWDGE + gpsimd SWDGE).
  * -r and 1/den staged to DRAM h-major (PE transpose -> one flat row
    write); R broadcast needs only 2 stride-0 DMAs, 1/den 8.
  * Q[j,i] = exp(|i-j| * -r) via one bf16 DVE multiply per (chunk,hp,hh)
    and ONE [128, 2048] ScalarE Exp per (chunk, hp).
  * AV bf16, 2 heads / psum tile; 4 blocks share a [128,512] psum bank so
    normalization is one [128,512] DVE op per (chunk, hp). out^T lands in
    the lhsT layout of the W_out projection; proj+bias+store per block.

Sharding: 8 cores = 4 batches x 2 sequence halves; no collectives.
"""

import numpy as np
import ml_dtypes

import concourse.bass as bass
import concourse.mybir as mybir
import concourse.tile as tile
from concourse import bacc
from concourse.bass_utils import run_bass_kernel_spmd

F32 = mybir.dt.float32
BF16 = mybir.dt.bfloat16

B, N, D = 4, 2048, 512
H, DH = 8, 64
HALF = N // 2            # 1024 rows per core
PAD = 128                # zero-pad rows at each end of the j range
NJROWS = HALF + 2 * PAD  # 1280 padded j rows per core
NBLK = HALF // 128       # 8 i-blocks per core
NVT = 9                  # V tiles at odd 64-offsets (rows 64k..64k+128, k odd)
CB = 4                   # i-blocks per chunk
NCH = NBLK // CB         # chunks

_nc_cache = None


def _build_nc():
    nc = bacc.Bacc("TRN2", target_bir_lowering=False, debug=False)

    # --- packed DRAM inputs ---
    xt01 = nc.dram_tensor("xt01", [128, 2 * NJROWS], BF16, kind="ExternalInput")
    xt23 = nc.dram_tensor("xt23", [128, 2 * NJROWS], BF16, kind="ExternalInput")
    wvb4 = nc.dram_tensor("wvb4", [128, 4 * D], BF16, kind="ExternalInput")
    wob4 = nc.dram_tensor("wob4", [128, 4 * D], BF16, kind="ExternalInput")
    wsb4 = nc.dram_tensor("wsb4", [128, 4 * H], BF16, kind="ExternalInput")
    # cpak_bf: m2r [128,256] | identity [128,128]
    cpak_bf = nc.dram_tensor("cpak_bf", [128, 384], BF16, kind="ExternalInput")
    # cpak_f32: bsig [128,8] | ivp1 [128,64] | ivnm [128,64]
    cpak_f32 = nc.dram_tensor("cpak_f32", [128, 136], F32, kind="ExternalInput")
    bout = nc.dram_tensor("bout", [128, D], F32, kind="ExternalInput")
    out = nc.dram_tensor("out", [HALF, D], F32, kind="ExternalOutput")
    # staging: negr / inv, h-major rows (row = h*8 + b), 128 i's per row
    negr_d = nc.dram_tensor("negr_d", [64, 128], BF16)
    inv_d = nc.dram_tensor("inv_d", [64, 128], BF16)

    EXP = mybir.ActivationFunctionType.Exp
    MUL = mybir.AluOpType.mult
    ADD = mybir.AluOpType.add

    with tile.TileContext(nc) as tc:
        with (
            tc.tile_pool(name="const", bufs=1) as cpool,
            tc.tile_pool(name="vpool", bufs=1) as vpool,
            tc.tile_pool(name="otpool", bufs=1) as otpool,
            tc.tile_pool(name="sg", bufs=1) as sgpool,
            tc.tile_pool(name="bc", bufs=1) as bcpool,
            tc.tile_pool(name="warm", bufs=1) as wpool,
        ):
            # ---- t=0: exp-table preload + HAM warmup (no data deps) ----
            dum = wpool.tile([128, 128], BF16, tag="dum")
            nc.vector.memset(dum[:], 0.25)
            dume = wpool.tile([128, 8], F32, tag="dume")
            nc.scalar.activation(dume[:], dum[:, 0:8], EXP)
            with tc.tile_pool(name="pswarm", bufs=1, space="PSUM") as psw:
                pw = psw.tile([128, 128], F32, tag="pw")
                for _ in range(16):
                    nc.tensor.matmul(pw[:], lhsT=dum[:], rhs=dum[:],
                                     start=True, stop=True)

            # ---------------- loads (3 engines, priority order) ---------
            xt01_t = cpool.tile([128, 2 * NJROWS], BF16, tag="xt01")
            nc.sync.dma_start(xt01_t[:], xt01[:, :])
            xt23_t = cpool.tile([128, 2 * NJROWS], BF16, tag="xt23")
            nc.scalar.dma_start(xt23_t[:], xt23[:, :])
            wsb_t = cpool.tile([128, 4 * H], BF16, tag="wsb")
            nc.gpsimd.dma_start(wsb_t[:], wsb4[:, :])
            cbf_t = cpool.tile([128, 384], BF16, tag="cbf")
            nc.sync.dma_start(cbf_t[:], cpak_bf[:, :])
            cf32_t = cpool.tile([128, 136], F32, tag="cf32")
            nc.sync.dma_start(cf32_t[:], cpak_f32[:, :])
            wvb_t = cpool.tile([128, 4 * D], BF16, tag="wvb")
            nc.scalar.dma_start(wvb_t[:], wvb4[:, :])
            wob_t = cpool.tile([128, 4 * D], BF16, tag="wob")
            nc.gpsimd.dma_start(wob_t[:], wob4[:, :])
            bout_t = cpool.tile([128, D], F32, tag="bout")
            nc.gpsimd.dma_start(bout_t[:], bout[:, :])

            m2r_t = cbf_t[:, 0:256]
            identb = cbf_t[:, 256:384]
            bsig = cf32_t[:, 0:8]
            ivp1 = cf32_t[:, 8:72]
            ivnm = cf32_t[:, 72:136]

            def xts(dt, c0, c1):
                t = xt01_t if dt < 2 else xt23_t
                off = (dt % 2) * NJROWS
                return t[:, off + c0:off + c1]

            # ------------- sigma GEMM (first PE work after warmup) -------
            with tc.tile_pool(name="pss", bufs=1, space="PSUM") as pss:
                ps = pss.tile([128, NBLK * H], F32, tag="ps")
                for b in range(NBLK):
                    for dt in range(4):
                        nc.tensor.matmul(
                            ps[:, b * H:(b + 1) * H],
                            lhsT=xts(dt, PAD + b * 128, PAD + (b + 1) * 128),
                            rhs=wsb_t[:, dt * H:(dt + 1) * H],
                            start=(dt == 0),
                            stop=(dt == 3),
                        )
                s_all = sgpool.tile([128, NBLK * H], F32, tag="s_all")
                nc.vector.tensor_tensor(
                    s_all[:].rearrange("p (b h) -> p b h", h=H),
                    ps[:].rearrange("p (b h) -> p b h", h=H),
                    bsig.rearrange("p (one h) -> p one h", one=1)
                    .broadcast_to((128, NBLK, H)),
                    op=ADD,
                )

            # ---- sigma chain without Sigmoid tables ----
            # sigma = 1/(1 + exp(-s));  den = 1 + exp(sigma);  r = 1/den
            ems = sgpool.tile([128, NBLK * H], F32, tag="ems")
            nc.scalar.activation(ems[:], s_all[:], EXP, scale=-1.0)
            d1 = sgpool.tile([128, NBLK * H], F32, tag="d1")
            nc.vector.tensor_scalar(d1[:], ems[:], 1.0, None, ADD)
            sig = sgpool.tile([128, NBLK * H], F32, tag="sig")
            nc.vector.reciprocal(sig[:], d1[:])
            esg = sgpool.tile([128, NBLK * H], F32, tag="esg")
            nc.scalar.activation(esg[:], sig[:], EXP)
            den = sgpool.tile([128, NBLK * H], F32, tag="den")
            nc.vector.tensor_scalar(den[:], esg[:], 1.0, None, ADD)
            r_all = sgpool.tile([128, NBLK * H], F32, tag="r_all")
            nc.vector.reciprocal(r_all[:], den[:])

            # negr (bf16, h-major cols h*8+b) -- stage ASAP, it gates AV
            negr_b = sgpool.tile([128, NBLK * H], BF16, tag="negr_b")
            nc.vector.tensor_scalar(
                negr_b[:].rearrange("p (h b) -> p h b", b=NBLK),
                r_all[:].rearrange("p (b h) -> p h b", h=H),
                -1.0, None, MUL,
            )
            with tc.tile_pool(name="pst", bufs=2, space="PSUM") as pst:
                ptn = pst.tile([64, 128], BF16, tag="ptn")
                nc.tensor.transpose(ptn[:], negr_b[:], identb)
                negrT = sgpool.tile([64, 128], BF16, tag="negrT")
                nc.scalar.copy(negrT[:], ptn[:])
                nc.sync.dma_start(negr_d.ap(), negrT[:, :])

                # ---- R_all broadcast: 2 fat stride-0 DMAs ----
                R_all = bcpool.tile([128, H * HALF], BF16, tag="R_all")
                nc.sync.dma_start(
                    R_all[:, 0:4 * HALF],
                    negr_d.ap().rearrange("r p -> (r p)").unsqueeze(0)
                    [:, 0:4 * HALF].to_broadcast((128, 4 * HALF)),
                )
                nc.scalar.dma_start(
                    R_all[:, 4 * HALF:8 * HALF],
                    negr_d.ap().rearrange("r p -> (r p)").unsqueeze(0)
                    [:, 4 * HALF:8 * HALF].to_broadcast((128, 4 * HALF)),
                )

                # ---- 1/den closed form: inv = w / (w + 2z - A - B), w=1-z
                z = sgpool.tile([128, NBLK * H], F32, tag="z")
                nc.scalar.activation(z[:], r_all[:], EXP, scale=-1.0)
                argA = sgpool.tile([128, NBLK * H], F32, tag="argA")
                nc.vector.tensor_mul(argA[:], r_all[:], ivp1)
                expA = sgpool.tile([128, NBLK * H], F32, tag="expA")
                nc.scalar.activation(expA[:], argA[:], EXP)
                argB = sgpool.tile([128, NBLK * H], F32, tag="argB")
                nc.vector.tensor_mul(argB[:], r_all[:], ivnm)
                expB = sgpool.tile([128, NBLK * H], F32, tag="expB")
                nc.scalar.activation(expB[:], argB[:], EXP)
                w = sgpool.tile([128, NBLK * H], F32, tag="w")
                nc.vector.tensor_scalar(w[:], z[:], -1.0, 1.0, MUL, ADD)
                t1 = sgpool.tile([128, NBLK * H], F32, tag="t1")
                nc.vector.tensor_scalar_mul(t1[:], z[:], 2.0)
                nc.vector.tensor_sub(t1[:], t1[:], expA[:])
                nc.vector.tensor_sub(t1[:], t1[:], expB[:])
                u = sgpool.tile([128, NBLK * H], F32, tag="u")
                nc.vector.tensor_add(u[:], w[:], t1[:])
                ru = sgpool.tile([128, NBLK * H], F32, tag="ru")
                nc.vector.reciprocal(ru[:], u[:])
                inv_c = sgpool.tile([128, NBLK * H], F32, tag="inv_c")
                nc.vector.tensor_mul(inv_c[:], w[:], ru[:])
                inv_b = sgpool.tile([128, NBLK * H], BF16, tag="inv_b")
                nc.vector.tensor_copy(
                    inv_b[:].rearrange("p (h b) -> p h b", b=NBLK),
                    inv_c[:].rearrange("p (b h) -> p h b", h=H),
                )
                pti = pst.tile([64, 128], BF16, tag="pti")
                nc.tensor.transpose(pti[:], inv_b[:], identb)
                invT = sgpool.tile([64, 128], BF16, tag="invT")
                nc.scalar.copy(invT[:], pti[:])
                nc.scalar.dma_start(inv_d.ap(), invT[:, :])

            # Iv_pair[p, hp*HALF + i] = 1/den[2*hp + (p>=64), i]
            Iv_pair = bcpool.tile([128, 4 * HALF], BF16, tag="Iv_pair")
            for hp in range(4):
                eng = nc.sync if hp % 2 == 0 else nc.scalar
                eng.dma_start(
                    Iv_pair[0:64, hp * HALF:(hp + 1) * HALF],
                    inv_d.ap().rearrange("r p -> (r p)").unsqueeze(0)
                    [:, (2 * hp) * 8 * 128:(2 * hp + 1) * 8 * 128]
                    .to_broadcast((64, HALF)),
                )
                eng.dma_start(
                    Iv_pair[64:128, hp * HALF:(hp + 1) * HALF],
                    inv_d.ap().rearrange("r p -> (r p)").unsqueeze(0)
                    [:, (2 * hp + 1) * 8 * 128:(2 * hp + 2) * 8 * 128]
                    .to_broadcast((64, HALF)),
                )

            # ---------------- V = x @ W_v (9 tiles at odd 64-offsets) ----
            V_t = []
            with tc.tile_pool(name="psv", bufs=3, space="PSUM") as psv:
                for vt_i in range(NVT):
                    k = 2 * vt_i + 1
                    pv = psv.tile([128, D], F32, tag="pv")
                    for dt in range(4):
                        nc.tensor.matmul(
                            pv[:],
                            lhsT=xts(dt, 64 * k, 64 * k + 128),
                            rhs=wvb_t[:, dt * D:(dt + 1) * D],
                            start=(dt == 0),
                            stop=(dt == 3),
                        )
                    vt = vpool.tile([128, D], BF16, tag=f"V{vt_i}")
                    if vt_i % 3 == 0:
                        nc.scalar.copy(vt[:], pv[:])
                    else:
                        nc.vector.tensor_copy(vt[:], pv[:])
                    V_t.append(vt)

            # persistent out^T tiles (bf16): tile hp = heads 2hp, 2hp+1
            outT_t = []
            for t in range(4):
                oT = otpool.tile([128, HALF], BF16, tag=f"oT{t}")
                outT_t.append(oT)

            # ---------------- main loop ----------------
            with (
                tc.tile_pool(name="qp", bufs=4) as qpool,
                tc.tile_pool(name="fin", bufs=3) as fpool,
                tc.tile_pool(name="psa", bufs=3, space="PSUM") as psa,
                tc.tile_pool(name="psf", bufs=2, space="PSUM") as psf,
            ):
                for ch in range(NCH):
                    for hp in range(4):
                        # ---- Q = exp(m2r * R): 2 DVE muls + ONE exp ----
                        Q = qpool.tile([128, 2 * CB * 256], BF16, tag="Q")
                        ARG = qpool.tile([128, 2 * CB * 256], BF16, tag="ARG")
                        for hh in range(2):
                            h = 2 * hp + hh
                            R = R_all[:, h * HALF + ch * CB * 128:
                                      h * HALF + (ch + 1) * CB * 128]
                            nc.vector.tensor_tensor(
                                ARG[:, hh * 1024:(hh + 1) * 1024]
                                .rearrange("p (b o q) -> p b o q", b=CB, o=2),
                                m2r_t
                                .rearrange("p (one o q) -> p one o q", one=1, o=2)
                                .broadcast_to((128, CB, 2, 128)),
                                R.rearrange("p (b one q) -> p b one q", b=CB, one=1)
                                .broadcast_to((128, CB, 2, 128)),
                                op=MUL,
                            )
                        nc.scalar.activation(Q[:], ARG[:], EXP)
                        # ---- AV: 4 blocks into one [128,512] psum ----
                        pav = psa.tile([128, CB * 128], F32, tag="pav")
                        for bi in range(CB):
                            b = ch * CB + bi
                            for hh in range(2):
                                h = 2 * hp + hh
                                for o in range(2):
                                    nc.tensor.matmul(
                                        pav[hh * 64:(hh + 1) * 64,
                                            bi * 128:(bi + 1) * 128],
                                        lhsT=V_t[b + o][:, h * 64:(h + 1) * 64],
                                        rhs=Q[:, hh * 1024 + bi * 256 + o * 128:
                                              hh * 1024 + bi * 256 + (o + 1) * 128],
                                        start=(o == 0),
                                        stop=(o == 1),
                                    )
                        # ---- normalize: ONE [128,512] DVE op ----
                        nc.vector.tensor_mul(
                            outT_t[hp][:, ch * 512:(ch + 1) * 512],
                            pav[:],
                            Iv_pair[:, hp * HALF + ch * 512:
                                    hp * HALF + (ch + 1) * 512],
                        )
                    # ---- projection + bias + store per block ----
                    for bi in range(CB):
                        b = ch * CB + bi
                        cols = slice(b * 128, (b + 1) * 128)
                        pf = psf.tile([128, D], F32, tag="pf")
                        for t in range(4):
                            nc.tensor.matmul(
                                pf[:],
                                lhsT=outT_t[t][:, cols],
                                rhs=wob_t[:, t * D:(t + 1) * D],
                                start=(t == 0),
                                stop=(t == 3),
                            )
                        fin = fpool.tile([128, D], F32, tag="fin")
                        nc.vector.tensor_add(fin[:], pf[:], bout_t[:])
                        eng = nc.sync if b % 2 == 0 else nc.scalar
                        eng.dma_start(out[cols, :], fin[:])

    nc.compile()
    return nc


def _make_in_maps(x, W_v, W_sigma, b_sigma, W_out, b_out):
    bf = ml_dtypes.bfloat16
    m2r1 = np.empty((128, 256), dtype=np.float32)
    p = np.arange(128, dtype=np.float32)[:, None]
    q = np.arange(128, dtype=np.float32)[None, :]
    for o in range(2):
        m2r1[:, o * 128:(o + 1) * 128] = np.abs(q - p + 64.0 - 128.0 * o)
    identb = np.eye(128, dtype=np.float32)
    cpak_bf = np.concatenate([m2r1, identb], axis=1).astype(bf)

    WvT = W_v.astype(bf)           # [512, 512] rows = d
    WsT = W_sigma.astype(bf)       # [512, 8]
    WoT = W_out.astype(bf)         # [512, 512]
    wvb4 = np.concatenate([WvT[i * 128:(i + 1) * 128] for i in range(4)],
                          axis=1)  # [128, 2048]
    wob4 = np.concatenate([WoT[i * 128:(i + 1) * 128] for i in range(4)],
                          axis=1)
    wsb4 = np.concatenate([WsT[i * 128:(i + 1) * 128] for i in range(4)],
                          axis=1)  # [128, 32]
    bsig_b = np.broadcast_to(b_sigma[None, :], (128, H)).astype(np.float32)
    bout_b = np.broadcast_to(b_out[None, :], (128, D)).copy().astype(np.float32)

    in_maps = []
    for c in range(8):
        bb, half = c // 2, c % 2
        i_start = half * HALF
        xp = np.zeros((NJROWS, D), dtype=np.float32)
        j_lo = max(0, i_start - PAD)
        j_hi = min(N, i_start + HALF + PAD)
        xp[j_lo - (i_start - PAD):j_hi - (i_start - PAD)] = x[bb, j_lo:j_hi]
        xT = np.ascontiguousarray(xp.T.astype(bf))     # [512, 1280]
        xt01 = np.concatenate([xT[0:128], xT[128:256]], axis=1)
        xt23 = np.concatenate([xT[256:384], xT[384:512]], axis=1)

        pcol = np.arange(128, dtype=np.float32)[:, None]
        blk = np.arange(NBLK, dtype=np.float32)[None, :]
        i_abs = i_start + blk * 128 + pcol                     # [128, NBLK]
        ivp1 = np.repeat(-(i_abs + 1.0), H, axis=1).astype(np.float32)
        ivnm = np.repeat(-(float(N) - i_abs), H, axis=1).astype(np.float32)
        cpak_f32 = np.concatenate([bsig_b, ivp1, ivnm], axis=1)

        in_maps.append(
            {
                "xt01": np.ascontiguousarray(xt01),
                "xt23": np.ascontiguousarray(xt23),
                "wvb4": np.ascontiguousarray(wvb4),
                "wob4": np.ascontiguousarray(wob4),
                "wsb4": np.ascontiguousarray(wsb4),
                "cpak_bf": np.ascontiguousarray(cpak_bf),
                "cpak_f32": np.ascontiguousarray(cpak_f32),
                "bout": bout_b,
            }
        )
    return in_maps


def kernel(x, W_v, W_sigma, b_sigma, W_out, b_out):
    global _nc_cache
    x = np.asarray(x, dtype=np.float32)
    W_v = np.asarray(W_v, dtype=np.float32)
    W_sigma = np.asarray(W_sigma, dtype=np.float32)
    b_sigma = np.asarray(b_sigma, dtype=np.float32)
    W_out = np.asarray(W_out, dtype=np.float32)
    b_out = np.asarray(b_out, dtype=np.float32)

    if _nc_cache is None:
        _nc_cache = _build_nc()
    nc = _nc_cache

    in_maps = _make_in_maps(x, W_v, W_sigma, b_sigma, W_out, b_out)
    res = run_bass_kernel_spmd(nc, in_maps, core_ids=list(range(8)))

    out = np.empty((B, N, D), dtype=np.float32)
    for c in range(8):
        bb, half = c // 2, c % 2
        out[bb, half * HALF:(half + 1) * HALF, :] = res.results[c]["out"]
    return out
